# revision 12
# baseline (speedup 1.0000x reference)
"""Trainium2 Bass kernel for multi-head attention (B=4, N=2048, C=768, H=12).

Sharding: 8 cores = 4 batches x 2 head-halves. Each core computes Q/K/V and
attention for its 6 heads (3 head-pairs) over the full 2048-token sequence,
then the final projection restricted to its 384 feature columns, producing a
partial [2048, 768] output. The host sums the two partials per batch (the
even core folds in the bias). No duplicated projection work, no collectives.

All matmul operands are fp16 (1 cyc/row on the PE; fp32 runs a 2x-slower
2-pass HIGH mode), accumulation is f32 in PSUM, softmax normalization in f32.

The kernel is paced by ScalarE exp (~25M exps/core is a hard floor). Emission
order = scheduler priority, so attention(pair 0) is emitted right after
pair 0's K/Q projection (x is DMA'd in 4 chunks so the first matmuls start
early); the V projection and later pairs' K/Q projections are emitted after
the first attention block and fill PE gaps in the ACT-paced stream, gated
only by their data dependencies. Attention per (pair, 512-query block ib,
128-key tile jt):
  ss[j, i]   = kT_h.T @ qT_h     (2 heads row-tiled, concurrent on the PE)
  et         = exp(SCALE*ss)     (ScalarE, PSUM->SBUF fp16, scale folded)
  po[0:65,i] += v'[j,0:65].T @ et  (v' = [v_h | ones]; row 64 = softmax denom)
normalize: outT = po[0:64] * rb where rb = ones[64] (x) 1/den, computed as a
PE outer product into PSUM (partition-broadcast without GpSimd). The two
denominators of a pair sit at partitions 0/32 of one tile (tiny SBUF DMAs)
so one DVE reciprocal covers both. The final projection for query block ib
runs right after the last pair's normalization of ib, hiding under the
remaining attention stream; only the last block's projection is a tail.
"""

import numpy as np

B, N, C = 4, 2048, 768
H, HD = 12, 64
SCALE = HD ** -0.5
P = 128
CT = C // P          # 6 contraction tiles for QKV projections
HC = C // 2          # 384 feature columns per core
PCT = HC // P        # 3 contraction tiles for the final projection
PAIRS = 3            # head pairs per core
JT = N // P          # 16 key tiles
IB = N // 512        # 4 query blocks
TKB = 512            # token-block width of projection matmuls
NCORES = 8

_cache = {}


def _build_bass():
    import concourse.bass as bass
    import concourse.tile as tile
    import concourse.mybir as mybir
    from concourse import bacc
    from concourse.bass import ts, ds
    from contextlib import ExitStack

    fr = mybir.dt.float32r
    f32 = mybir.dt.float32
    f16 = mybir.dt.float16
    Exp = mybir.ActivationFunctionType.Exp

    nc = bacc.Bacc("TRN2", target_bir_lowering=False, debug=False)

    xt_d = nc.dram_tensor("xt", [C, N], f16, kind="ExternalInput").ap()
    wq_d = nc.dram_tensor("wq", [C, HC], f16, kind="ExternalInput").ap()
    wk_d = nc.dram_tensor("wk", [C, HC], f16, kind="ExternalInput").ap()
    wv_d = nc.dram_tensor("wv", [C, HC], f16, kind="ExternalInput").ap()
    wp_d = nc.dram_tensor("wp", [HC, C], f16, kind="ExternalInput").ap()
    bb_d = nc.dram_tensor("bb", [P, C], f32, kind="ExternalInput").ap()
    out_d = nc.dram_tensor("out", [N, C], f32, kind="ExternalOutput").ap()

    xt_r = xt_d.rearrange("(o p) n -> p o n", p=P)
    wq_r = wq_d.rearrange("(o p) n -> p o n", p=P)
    wk_r = wk_d.rearrange("(o p) n -> p o n", p=P)
    wv_r = wv_d.rearrange("(o p) n -> p o n", p=P)
    wp_r = wp_d.rearrange("(o p) n -> p o n", p=P)
    out_r = out_d.rearrange("(t p) n -> t p n", p=P)

    with tile.TileContext(nc) as tc:
        with ExitStack() as ctx:
            persist = ctx.enter_context(tc.tile_pool(name="persist", bufs=1))
            # full x kept resident: [128, 6 ctiles, 2048 tokens] fp16.
            # 4 chunked DMAs so the first projection matmuls start early.
            xt_sb = persist.tile([P, CT, N], f16, name="xt_sb")
            wk_sb = persist.tile([P, CT, HC], f16, name="wk_sb")
            nc.sync.dma_start(wk_sb[:], wk_r)
            wq_sb = persist.tile([P, CT, HC], f16, name="wq_sb")
            nc.sync.dma_start(wq_sb[:], wq_r)
            for tb in range(N // TKB):
                nc.sync.dma_start(
                    xt_sb[:, :, ts(tb, TKB)], xt_r[:, :, ts(tb, TKB)]
                )
            wv_sb = persist.tile([P, CT, HC], f16, name="wv_sb")
            nc.sync.dma_start(wv_sb[:], wv_r)
            wp_sb = persist.tile([P, PCT, C], f16, name="wp_sb")
            nc.sync.dma_start(wp_sb[:], wp_r)
            bias_sb = persist.tile([P, C], f32, name="bias_sb")
            nc.sync.dma_start(bias_sb[:], bb_d)

            # pair-packed K/Q: partitions 0:64 even head, 64:128 odd head
            kT_sb = persist.tile([P, PAIRS, N], f16, name="kT_sb")
            qT_sb = persist.tile([P, PAIRS, N], f16, name="qT_sb")
            # V + ones column: [keys 128, key-tile, head, 66] (col 64 = ones)
            v_all = persist.tile([P, JT, 6, 66], f16, name="v_all")
            outT_sb = persist.tile([P, PAIRS, N], f16, name="outT_sb")
            # ones rows at partitions 0 and 32 for the rb outer products
            ones_sb = persist.tile([33, 64], f16, name="ones_sb")
            # rows 0/32 receive each pair's two denominators; rows 1-31 only
            # feed wasted reciprocal lanes but must be nonzero and initialized
            den_q = persist.tile([33, 512], fr, name="den_q")
            with nc.allow_low_precision(reason="ones constant is exact in f16"):
                nc.vector.tensor_copy(
                    v_all[:, :, :, 64:66],
                    nc.const_aps.tensor(1.0, [P, JT, 6, 2], f32),
                )
                nc.vector.tensor_copy(
                    ones_sb[:], nc.const_aps.tensor(1.0, [33, 64], f32)
                )
                nc.vector.tensor_copy(
                    den_q[:], nc.const_aps.tensor(1.0, [33, 512], f32)
                )

            apsum = ctx.enter_context(
                tc.tile_pool(name="apsum", bufs=2, space="PSUM")
            )
            spsum = ctx.enter_context(
                tc.tile_pool(name="spsum", bufs=2, space="PSUM")
            )
            opsum = ctx.enter_context(
                tc.tile_pool(name="opsum", bufs=2, space="PSUM")
            )
            expt_pool = ctx.enter_context(tc.tile_pool(name="expt", bufs=6))
            nrm_pool = ctx.enter_context(tc.tile_pool(name="nrm", bufs=2))
            poS_pool = ctx.enter_context(tc.tile_pool(name="poSp", bufs=4))
            outsb_pool = ctx.enter_context(tc.tile_pool(name="outsb", bufs=2))

            def kq_proj(p, order="kq"):
                for tb in range(N // TKB):
                    for which in order:
                        w_sb, dst = (
                            (wk_sb, kT_sb) if which == "k" else (wq_sb, qT_sb)
                        )
                        ps = apsum.tile([P, TKB], f32, tag="aps")
                        for c in range(CT):
                            nc.tensor.matmul(
                                ps[:],
                                w_sb[:, c, ts(p, P)],
                                xt_sb[:, c, ts(tb, TKB)],
                                start=(c == 0),
                                stop=(c == CT - 1),
                            )
                        with nc.allow_low_precision(reason="f16 kq path"):
                            nc.vector.tensor_copy(dst[:, p, ts(tb, TKB)], ps[:])

            def v_tile(tt):
                # V projection, all 6 heads at once for one 128-token tile:
                # token-tile stationary, wv moving
                ps = apsum.tile([P, TKB], f32, tag="aps")
                for c in range(CT):
                    nc.tensor.matmul(
                        ps[:, 0:HC],
                        xt_sb[:, c, ts(tt, P)],
                        wv_sb[:, c, :],
                        start=(c == 0),
                        stop=(c == CT - 1),
                    )
                with nc.allow_low_precision(reason="f16 value path"):
                    nc.vector.tensor_copy(
                        v_all[:, tt, :, 0:64],
                        ps[:, 0:HC].rearrange("p (h e) -> p h e", e=64),
                    )

            def out_proj(ib):
                # final projection for the 4 token tiles of query block ib
                for g in range(4):
                    git = 4 * ib + g
                    ob = outsb_pool.tile([P, C], f32, tag="ob")
                    for n0, n1 in ((0, 512), (512, 768)):
                        pp = apsum.tile([P, TKB], f32, tag="aps")
                        for t in range(PAIRS):
                            nc.tensor.matmul(
                                pp[:, 0 : n1 - n0],
                                outT_sb[:, t, ds(git * P, P)],
                                wp_sb[:, t, n0:n1],
                                start=(t == 0),
                                stop=(t == PAIRS - 1),
                            )
                        nc.vector.tensor_add(
                            ob[:, n0:n1], pp[:, 0 : n1 - n0], bias_sb[:, n0:n1]
                        )
                    nc.sync.dma_start(out_r[git], ob[:])

            def attention(p, ib, with_v=False):
                po0 = opsum.tile([P, 512], f32, tag="po")
                po1 = opsum.tile([P, 512], f32, tag="po")
                pos = (po0, po1)
                for jt in range(JT):
                    ss = spsum.tile([P, 1024], f32, tag="ss")
                    nc.tensor.matmul(
                        ss[:, 0:512],
                        kT_sb[0:64, p, ts(jt, P)],
                        qT_sb[0:64, p, ts(ib, 512)],
                        start=True,
                        stop=True,
                    )
                    nc.tensor.matmul(
                        ss[:, 512:1024],
                        kT_sb[64:128, p, ts(jt, P)],
                        qT_sb[64:128, p, ts(ib, 512)],
                        start=True,
                        stop=True,
                    )
                    et = expt_pool.tile([P, 1024], f16, tag="et")
                    nc.scalar.activation(et[:], ss[:], Exp, scale=SCALE)
                    if with_v:
                        v_tile(jt)
                    for hh in range(2):
                        nc.tensor.matmul(
                            pos[hh][0:65, :],
                            v_all[:, jt, 2 * p + hh, 0:65],
                            et[:, hh * 512 : (hh + 1) * 512],
                            start=(jt == 0),
                            stop=(jt == JT - 1),
                        )
                poS_all = []
                for hh in range(2):
                    poS = poS_pool.tile([65, 512], fr, tag="poS")
                    with nc.allow_low_precision(reason="f32r is bitwise f32"):
                        nc.vector.tensor_copy(poS[:], pos[hh][0:65, :])
                    # stack this head's denominator at partition 32*hh
                    nc.sync.dma_start(
                        den_q[32 * hh : 32 * hh + 1, :], poS[64:65, :]
                    )
                    poS_all.append(poS)
                rd_q = nrm_pool.tile([33, 512], f16, tag="rd_q")
                with nc.allow_low_precision(reason="f16 reciprocal scale"):
                    nc.vector.reciprocal(rd_q[:], den_q[:])
                for hh in range(2):
                    # partition-broadcast 1/den via PE outer product:
                    # rb[0:64, q] = ones[64] * rd[q]
                    rb_ps = apsum.tile([P, TKB], f32, tag="aps")
                    nc.tensor.matmul(
                        rb_ps[0:64, :],
                        ones_sb[32 * hh : 32 * hh + 1, :],
                        rd_q[32 * hh : 32 * hh + 1, :],
                        start=True,
                        stop=True,
                    )
                    with nc.allow_low_precision(reason="f16 attn output"):
                        nc.vector.tensor_mul(
                            outT_sb[hh * 64 : (hh + 1) * 64, p, ts(ib, 512)],
                            poS_all[hh][0:64, :],
                            rb_ps[0:64, :],
                        )

            kq_proj(0)
            attention(0, 0, with_v=True)
            attention(0, 1)
            kq_proj(1, order="qk")
            attention(0, 2)
            attention(0, 3)
            attention(1, 0)
            kq_proj(2, order="qk")
            for ib in range(1, IB):
                attention(1, ib)
            for ib in range(IB):
                attention(2, ib)
                out_proj(ib)

    nc.compile()
    return nc


def _get_nc():
    if "nc" not in _cache:
        _cache["nc"] = _build_bass()
    return _cache["nc"]


def _prep_in_maps(x, w_qkv, w_proj, b_proj):
    x = np.asarray(x, np.float32)
    w_qkv = np.asarray(w_qkv, np.float32)
    w_proj = np.asarray(w_proj, np.float32)
    b_proj = np.asarray(b_proj, np.float32)

    wq = np.ascontiguousarray(w_qkv[0:C].T).astype(np.float16)
    wk = np.ascontiguousarray(w_qkv[C : 2 * C].T).astype(np.float16)
    wv = np.ascontiguousarray(w_qkv[2 * C : 3 * C].T).astype(np.float16)
    wp = np.ascontiguousarray(w_proj.T).astype(np.float16)
    bb = np.ascontiguousarray(np.broadcast_to(b_proj[None, :], (P, C)))
    zb = np.zeros((P, C), np.float32)

    in_maps = []
    for core in range(NCORES):
        b, half = core // 2, core % 2
        xt = np.ascontiguousarray(x[b].T).astype(np.float16)
        sl = slice(half * HC, (half + 1) * HC)
        in_maps.append(
            {
                "xt": xt,
                "wq": np.ascontiguousarray(wq[:, sl]),
                "wk": np.ascontiguousarray(wk[:, sl]),
                "wv": np.ascontiguousarray(wv[:, sl]),
                "wp": np.ascontiguousarray(wp[sl, :]),
                "bb": bb if half == 0 else zb,
            }
        )
    return in_maps


def run(x, w_qkv, w_proj, b_proj, trace=False):
    from concourse import bass_utils

    nc = _get_nc()
    in_maps = _prep_in_maps(x, w_qkv, w_proj, b_proj)
    br = bass_utils.run_bass_kernel_spmd(
        nc, in_maps, core_ids=list(range(NCORES)), trace=trace
    )
    y = np.empty((B, N, C), np.float32)
    for b in range(B):
        y[b] = br.results[2 * b]["out"]
        y[b] += br.results[2 * b + 1]["out"]
    return y, br


def kernel(x, w_qkv, w_proj, b_proj):
    y, _ = run(x, w_qkv, w_proj, b_proj, trace=False)
    return y
